# revision 11
# baseline (speedup 1.0000x reference)
"""Kendall-tau loss kernel v2 for Trainium2 (Bass/Tile), 8-core SPMD.

v2: triangle-split pair counting — roughly halves ACT+DVE work vs the
full ordered-pair sweep and spreads it over three engines.

Per row (N=2048, 16 chunks of 128):
  For a-chunk c (a on partitions), window w = [128c, N):
    ACT:   sp = Sign(p_bcast + bias(-p_a)) on the window,
           accum_out -> per-partition sum of sign(p_b - p_a).
           (Over the diagonal block the sign sum is antisymmetric and
           totals 0, so the window accum == upper-block sign sum.)
    GPSIMD: diag STT on sp[:, :128]:  [t_b > t_a] * sp  -> dacc
    DVE:    upper STT on sp[:, 128:]: [t_b > t_a] * sp  -> uacc
  S_row = sum(dacc) + 2*sum(uacc) - sum(sacc):
    diag pairs counted once via t-ascending orientation ([t>]*sign);
    upper pairs (each evaluated once) via sign(td)*sign(pd)
    = (2[t_b>t_a]-1)*sign(pd)  (t-rounding-ties add ~1e-6 rel noise).
  tau = S / (N(N-1)), loss = 1 - mean(tau).  All partial sums are
  integers < 2^24 -- exact in f32.

Inputs ride in as fp16 (order-preserving rounding, ~1e-6 rel effect,
half the upload).  Dispatch: cached fast_dispatch_compile executable,
one blocking sync per call (see kernel.py docstring for the axon
latency model).
"""

import os
import numpy as np

N = 2048
P = 128
NCHUNK = N // P  # 16
ROWS_PER_CORE = 4
N_CORES = 8
PAIRS = float(N * (N - 1) // 2)

_cache = {}


def _patch_tile_drain():
    """The walrus build in this container rejects sync-waits on CTRL
    instructions (Drain/NOP): replace TileContext's kernel-tail
    drain-with-waits by a chain of single-wait event-semaphore
    instructions followed by a bare drain."""
    import concourse.mybir as mybir
    from concourse.tile import TileContext, ScopedClock

    if getattr(TileContext, "_ktau_drain_patched", False):
        return

    def _drain_and_barrier(self, tick_clock, wait_clock):
        tmp = self.nc.sync.nop()
        wait_clock.add_sem_waits(
            tmp.ins, ScopedClock({None: tick_clock.global_clock})
        )
        waits = list(tmp.ins.sync_info.on_wait)
        tmp.ins.sync_info = mybir.SyncInfo(
            on_update=list(tmp.ins.sync_info.on_update), on_wait=[]
        )
        num2handle = {h.num: h for h in self.sems.allocated().values()}
        for w in waits:
            self.nc.sync.wait_ge(num2handle[w.id], w.wait_value)
        self.nc.sync.drain()
        self.nc.all_engine_barrier()
        popped = self.nc._tile_sem_poison_stack.pop()
        assert popped is self._sem_poison
        self.nc.clear_and_free_semaphores(list(self.sems.allocated().values()))
        self.nc.all_engine_barrier()

    TileContext._drain_and_barrier = _drain_and_barrier
    TileContext._ktau_drain_patched = True


def _split_waits(nc, max_waits=1):
    """This container's walrus encodes at most one sem-wait per
    instruction; hoist excess waits onto single-wait EventSemaphore
    instructions inserted just before the consumer on the same engine."""
    import concourse.mybir as mybir

    n = 0
    for fn in nc.m.functions:
        for bb in fn.blocks:
            new_list = []
            for ins in bb.instructions:
                si = ins.sync_info
                waits = list(si.on_wait) if si is not None else []
                if len(waits) > max_waits:
                    for w in waits[:-max_waits]:
                        n += 1
                        ev = mybir.InstEventSemaphore(
                            name=f"WSPLIT-{n}",
                            engine=ins.engine,
                            sync_info=mybir.SyncInfo(on_update=[], on_wait=[w]),
                        )
                        new_list.append(ev)
                    ins.sync_info = mybir.SyncInfo(
                        on_update=list(si.on_update), on_wait=waits[-max_waits:]
                    )
                new_list.append(ins)
            bb.instructions = new_list


def _build_nc():
    import concourse.bass as bass
    import concourse.mybir as mybir
    import concourse.tile as tile

    _patch_tile_drain()
    f32 = mybir.dt.float32
    f16 = mybir.dt.float16

    nc = bass.Bass("TRN2")
    # merged input: rows 0..3 = pred, rows 4..7 = target (fp16)
    x_in = nc.dram_tensor("x", [2 * ROWS_PER_CORE, N], f16, kind="ExternalInput")
    q_out = nc.dram_tensor("q", [1, 3], f32, kind="ExternalOutput")

    with tile.TileContext(nc) as tc:
        with (
            tc.tile_pool(name="bcast", bufs=2) as bpool,
            tc.tile_pool(name="cols", bufs=2) as cpool,
            tc.tile_pool(name="sig", bufs=6) as sigpool,
            tc.tile_pool(name="scr", bufs=2) as spool,
            tc.tile_pool(name="acc", bufs=1) as apool,
        ):
            dacc = apool.tile([P, ROWS_PER_CORE * NCHUNK], f32)        # diag STT
            uacc = apool.tile([P, ROWS_PER_CORE * (NCHUNK - 1)], f32)  # upper STT
            sacc = apool.tile([P, ROWS_PER_CORE * NCHUNK], f32)        # ACT sign sums
            for r in range(ROWS_PER_CORE):
                pb = bpool.tile([P, N], f16, tag="pb")
                tb = bpool.tile([P, N], f16, tag="tb")
                nc.sync.dma_start(pb[:], x_in[r : r + 1, :].to_broadcast((P, N)))
                nc.sync.dma_start(
                    tb[:],
                    x_in[ROWS_PER_CORE + r : ROWS_PER_CORE + r + 1, :].to_broadcast(
                        (P, N)
                    ),
                )
                # p_cols[i, c] = p[128c + i]
                pc = cpool.tile([P, NCHUNK], f16, tag="pc")
                tcl = cpool.tile([P, NCHUNK], f16, tag="tc")
                nc.sync.dma_start(pc[:], x_in[r, :].rearrange("(c p) -> p c", p=P))
                nc.sync.dma_start(
                    tcl[:], x_in[ROWS_PER_CORE + r, :].rearrange("(c p) -> p c", p=P)
                )
                npc = cpool.tile([P, NCHUNK], f16, tag="npc")
                nc.gpsimd.tensor_scalar(
                    npc[:], pc[:], -1.0, None, mybir.AluOpType.mult
                )
                for c in range(NCHUNK):
                    lo = P * c
                    w = N - lo  # window width: diag block + all later b
                    # sp = sign(p_b - p_a) for b in [128c, N); the accum
                    # column collects sum_b sign(p_b - p_a), whose diag-
                    # block part cancels exactly (antisymmetric).
                    sp = sigpool.tile([P, N], f16, tag="sp")
                    nc.scalar.activation(
                        sp[:, :w], pb[:, lo:],
                        mybir.ActivationFunctionType.Sign,
                        bias=npc[:, c : c + 1], scale=1.0,
                        accum_out=sacc[:, r * NCHUNK + c : r * NCHUNK + c + 1],
                    )
                    # diag block (TensorScalarPtr only encodes on DVE)
                    scrd = spool.tile([P, P], f16, tag="scrd")
                    nc.vector.scalar_tensor_tensor(
                        scrd[:],
                        tb[:, lo : lo + P],
                        tcl[:, c : c + 1],
                        sp[:, :P],
                        mybir.AluOpType.is_gt,
                        mybir.AluOpType.mult,
                        accum_out=dacc[:, r * NCHUNK + c : r * NCHUNK + c + 1],
                    )
                    # strictly-upper part on DVE
                    if c < NCHUNK - 1:
                        scr = spool.tile([P, N], f16, tag="scr")
                        nc.vector.scalar_tensor_tensor(
                            scr[:, : w - P],
                            tb[:, lo + P :],
                            tcl[:, c : c + 1],
                            sp[:, P:w],
                            mybir.AluOpType.is_gt,
                            mybir.AluOpType.mult,
                            accum_out=uacc[
                                :, r * (NCHUNK - 1) + c : r * (NCHUNK - 1) + c + 1
                            ],
                        )
            # on-device reduction to [1,3]: (sum dacc, sum uacc, sum sacc)
            red = apool.tile([P, 3], f32)
            nc.vector.tensor_reduce(
                red[:, 0:1], dacc[:], mybir.AxisListType.X, mybir.AluOpType.add
            )
            nc.vector.tensor_reduce(
                red[:, 1:2], uacc[:], mybir.AxisListType.X, mybir.AluOpType.add
            )
            nc.vector.tensor_reduce(
                red[:, 2:3], sacc[:], mybir.AxisListType.X, mybir.AluOpType.add
            )
            q = apool.tile([1, 3], f32)
            nc.gpsimd.tensor_reduce(
                q[:], red[:], mybir.AxisListType.C, mybir.AluOpType.add
            )
            nc.sync.dma_start(q_out[:], q[:])
    _split_waits(nc)
    return nc


def _get_nc():
    if "nc" not in _cache:
        _cache["nc"] = _build_nc()
    return _cache["nc"]


def _build_fast():
    """AOT-compile the 8-core sharded executable once (fast C++ dispatch)."""
    import jax
    import concourse.mybir as mybir
    from jax.sharding import Mesh, PartitionSpec

    try:
        from jax.experimental.shard_map import shard_map
    except ImportError:
        from jax import shard_map

    from concourse.bass2jax import (
        _bass_exec_p,
        install_neuronx_cc_hook,
        partition_id_tensor,
        fast_dispatch_compile,
    )

    install_neuronx_cc_hook()
    nc = _get_nc()
    partition_name = nc.partition_id_tensor.name if nc.partition_id_tensor else None

    in_names, out_names, out_avals, zero_outs = [], [], [], []
    for alloc in nc.m.functions[0].allocations:
        if not isinstance(alloc, mybir.MemoryLocationSet):
            continue
        name = alloc.memorylocations[0].name
        if alloc.kind == "ExternalInput":
            if name != partition_name:
                in_names.append(name)
        elif alloc.kind == "ExternalOutput":
            out_names.append(name)
            shape = tuple(alloc.tensor_shape)
            dtype = mybir.dt.np(alloc.dtype)
            out_avals.append(jax.core.ShapedArray(shape, dtype))
            zero_outs.append(np.zeros(shape, dtype))
    n_params = len(in_names)
    n_outs = len(out_avals)
    in_names_all = list(in_names) + list(out_names)
    if partition_name is not None:
        in_names_all.append(partition_name)
    donate = tuple(range(n_params, n_params + n_outs))

    def _body(*args):
        operands = list(args)
        if partition_name is not None:
            operands.append(partition_id_tensor())
        outs = _bass_exec_p.bind(
            *operands,
            out_avals=tuple(out_avals),
            in_names=tuple(in_names_all),
            out_names=tuple(out_names),
            lowering_input_output_aliases=(),
            sim_require_finite=True,
            sim_require_nnan=True,
            nc=nc,
        )
        return tuple(outs)

    devices = jax.devices()[:N_CORES]
    mesh = Mesh(np.asarray(devices), ("core",))
    in_specs = (PartitionSpec("core"),) * (n_params + n_outs)
    out_specs = (PartitionSpec("core"),) * len(out_names)

    global_in = [
        jax.ShapeDtypeStruct((N_CORES * 2 * ROWS_PER_CORE, N), np.float16)
        for _ in range(n_params)
    ]
    global_zero = [
        jax.ShapeDtypeStruct((N_CORES * z.shape[0], *z.shape[1:]), z.dtype)
        for z in zero_outs
    ]

    compiled = fast_dispatch_compile(
        lambda: jax.jit(
            shard_map(
                _body,
                mesh=mesh,
                in_specs=in_specs,
                out_specs=out_specs,
                check_rep=False,
            ),
            donate_argnums=donate,
            keep_unused=True,
        )
        .lower(*global_in, *global_zero)
        .compile()
    )
    zero_shapes = [(N_CORES * z.shape[0], *z.shape[1:]) for z in zero_outs]
    zero_dtypes = [z.dtype for z in zero_outs]
    zero_meta = list(zip(zero_shapes, zero_dtypes))

    # warm the executable once (first device call pays extra runtime setup)
    dummy = np.zeros((N_CORES * 2 * ROWS_PER_CORE, N), np.float16)
    out = compiled(dummy, *[np.zeros(s, d) for s, d in zero_meta])
    np.asarray(out[0])

    return compiled, in_names, zero_meta


def _get_fast():
    if "fast" not in _cache:
        _cache["fast"] = _build_fast()
    return _cache["fast"]


def _loss_from_s(s_total, n_rows):
    return np.float32(1.0 - s_total / (n_rows * PAIRS))


def _pack_inputs(pred, target):
    x = np.empty((N_CORES, 2 * ROWS_PER_CORE, N), np.float16)
    x[:, :ROWS_PER_CORE] = pred.reshape(N_CORES, ROWS_PER_CORE, N)
    x[:, ROWS_PER_CORE:] = target.reshape(N_CORES, ROWS_PER_CORE, N)
    return x.reshape(N_CORES * 2 * ROWS_PER_CORE, N)


def _combine(q):
    """q: [..., 3] per-core (diag, upper_stt, sign_sum) -> total S."""
    q = np.asarray(q, np.float64).reshape(-1, 3)
    return float(q[:, 0].sum() + 2.0 * q[:, 1].sum() - q[:, 2].sum())


def _kernel_fallback(pred, target):
    from concourse.bass_utils import run_bass_kernel_spmd

    nc = _get_nc()
    xg = _pack_inputs(pred, target)
    in_maps = [
        {
            "x": np.ascontiguousarray(
                xg[k * 2 * ROWS_PER_CORE : (k + 1) * 2 * ROWS_PER_CORE]
            )
        }
        for k in range(N_CORES)
    ]
    res = run_bass_kernel_spmd(nc, in_maps, core_ids=list(range(N_CORES)))
    _cache["last_perf"] = res
    s_total = _combine(np.stack([r["q"] for r in res.results]))
    return _loss_from_s(s_total, pred.shape[0])


def kernel(pred, target):
    pred = np.asarray(pred, dtype=np.float32).reshape(-1, N)
    target = np.asarray(target, dtype=np.float32).reshape(-1, N)
    n_rows = pred.shape[0]
    assert n_rows == ROWS_PER_CORE * N_CORES

    if os.environ.get("KTAU_FALLBACK", "0") == "1":
        return _kernel_fallback(pred, target)
    try:
        compiled, in_names, zero_meta = _get_fast()
    except Exception:
        return _kernel_fallback(pred, target)

    xg = _pack_inputs(pred, target)
    zeros = [np.zeros(shape, dt) for shape, dt in zero_meta]
    out = compiled(xg, *zeros)
    s_total = _combine(np.asarray(out[0]))
    return _loss_from_s(s_total, n_rows)


# revision 14
# speedup vs baseline: 1.0188x; 1.0188x over previous
"""Kendall-tau loss kernel v2 for Trainium2 (Bass/Tile), 8-core SPMD.

v2: triangle-split pair counting — roughly halves ACT+DVE work vs the
full ordered-pair sweep and spreads it over three engines.

Per row (N=2048, 16 chunks of 128):
  For a-chunk c (a on partitions), window w = [128c, N):
    ACT:   sp = Sign(p_bcast + bias(-p_a)) on the window,
           accum_out -> per-partition sum of sign(p_b - p_a).
           (Over the diagonal block the sign sum is antisymmetric and
           totals 0, so the window accum == upper-block sign sum.)
    GPSIMD: diag STT on sp[:, :128]:  [t_b > t_a] * sp  -> dacc
    DVE:    upper STT on sp[:, 128:]: [t_b > t_a] * sp  -> uacc
  S_row = sum(dacc) + 2*sum(uacc) - sum(sacc):
    diag pairs counted once via t-ascending orientation ([t>]*sign);
    upper pairs (each evaluated once) via sign(td)*sign(pd)
    = (2[t_b>t_a]-1)*sign(pd)  (t-rounding-ties add ~1e-6 rel noise).
  tau = S / (N(N-1)), loss = 1 - mean(tau).  All partial sums are
  integers < 2^24 -- exact in f32.

Inputs ride in as fp16 (order-preserving rounding, ~1e-6 rel effect,
half the upload).  Dispatch: cached fast_dispatch_compile executable,
one blocking sync per call (see kernel.py docstring for the axon
latency model).
"""

import os
import numpy as np

N = 2048
P = 128
NCHUNK = N // P  # 16
ROWS_PER_CORE = 4
N_CORES = 8
PAIRS = float(N * (N - 1) // 2)

_cache = {}


def _patch_tile_drain():
    """The walrus build in this container rejects sync-waits on CTRL
    instructions (Drain/NOP): replace TileContext's kernel-tail
    drain-with-waits by a chain of single-wait event-semaphore
    instructions followed by a bare drain."""
    import concourse.mybir as mybir
    from concourse.tile import TileContext, ScopedClock

    if getattr(TileContext, "_ktau_drain_patched", False):
        return

    def _drain_and_barrier(self, tick_clock, wait_clock):
        tmp = self.nc.sync.nop()
        wait_clock.add_sem_waits(
            tmp.ins, ScopedClock({None: tick_clock.global_clock})
        )
        waits = list(tmp.ins.sync_info.on_wait)
        tmp.ins.sync_info = mybir.SyncInfo(
            on_update=list(tmp.ins.sync_info.on_update), on_wait=[]
        )
        num2handle = {h.num: h for h in self.sems.allocated().values()}
        for w in waits:
            self.nc.sync.wait_ge(num2handle[w.id], w.wait_value)
        self.nc.sync.drain()
        self.nc.all_engine_barrier()
        popped = self.nc._tile_sem_poison_stack.pop()
        assert popped is self._sem_poison
        self.nc.clear_and_free_semaphores(list(self.sems.allocated().values()))
        self.nc.all_engine_barrier()

    TileContext._drain_and_barrier = _drain_and_barrier
    TileContext._ktau_drain_patched = True


def _split_waits(nc, max_waits=1):
    """This container's walrus encodes at most one sem-wait per
    instruction; hoist excess waits onto single-wait EventSemaphore
    instructions inserted just before the consumer on the same engine."""
    import concourse.mybir as mybir

    n = 0
    for fn in nc.m.functions:
        for bb in fn.blocks:
            new_list = []
            for ins in bb.instructions:
                si = ins.sync_info
                waits = list(si.on_wait) if si is not None else []
                if len(waits) > max_waits:
                    for w in waits[:-max_waits]:
                        n += 1
                        ev = mybir.InstEventSemaphore(
                            name=f"WSPLIT-{n}",
                            engine=ins.engine,
                            sync_info=mybir.SyncInfo(on_update=[], on_wait=[w]),
                        )
                        new_list.append(ev)
                    ins.sync_info = mybir.SyncInfo(
                        on_update=list(si.on_update), on_wait=waits[-max_waits:]
                    )
                new_list.append(ins)
            bb.instructions = new_list


def _build_nc():
    import concourse.bass as bass
    import concourse.mybir as mybir
    import concourse.tile as tile

    _patch_tile_drain()
    f32 = mybir.dt.float32
    f16 = mybir.dt.float16

    nc = bass.Bass("TRN2")
    # merged input: rows 0..3 = pred, rows 4..7 = target (fp16)
    x_in = nc.dram_tensor("x", [2 * ROWS_PER_CORE, N], f16, kind="ExternalInput")
    q_out = nc.dram_tensor("q", [1, 3 * ROWS_PER_CORE], f32, kind="ExternalOutput")

    with tile.TileContext(nc) as tc:
        with (
            tc.tile_pool(name="bcast", bufs=2) as bpool,
            tc.tile_pool(name="cols", bufs=2) as cpool,
            tc.tile_pool(name="sig", bufs=6) as sigpool,
            tc.tile_pool(name="scr", bufs=2) as spool,
            tc.tile_pool(name="acc", bufs=2) as apool,
        ):
            # per-partition row partials reduced per-row into red so the
            # reduce tail overlaps later rows' compute
            red = apool.tile([P, 3 * ROWS_PER_CORE], f32)
            for r in range(ROWS_PER_CORE):
                dacc = apool.tile([P, NCHUNK], f32, tag="dacc")        # diag STT
                uacc = apool.tile([P, NCHUNK - 1], f32, tag="uacc")    # upper STT
                sacc = apool.tile([P, NCHUNK], f32, tag="sacc")        # ACT sign sums
                pb = bpool.tile([P, N], f16, tag="pb")
                tb = bpool.tile([P, N], f16, tag="tb")
                nc.sync.dma_start(pb[:], x_in[r : r + 1, :].to_broadcast((P, N)))
                nc.sync.dma_start(
                    tb[:],
                    x_in[ROWS_PER_CORE + r : ROWS_PER_CORE + r + 1, :].to_broadcast(
                        (P, N)
                    ),
                )
                # p_cols[i, c] = p[128c + i]
                pc = cpool.tile([P, NCHUNK], f16, tag="pc")
                tcl = cpool.tile([P, NCHUNK], f16, tag="tc")
                nc.sync.dma_start(pc[:], x_in[r, :].rearrange("(c p) -> p c", p=P))
                nc.sync.dma_start(
                    tcl[:], x_in[ROWS_PER_CORE + r, :].rearrange("(c p) -> p c", p=P)
                )
                for c in range(NCHUNK):
                    lo = P * c
                    w = N - lo  # window width: diag block + all later b
                    # sp = sign(p_a - p_b) = -sign(p_b - p_a) for b in
                    # [128c, N): Sign(pb * -1 + bias(p_a)) -- scale=-1
                    # avoids a gpsimd negation pass on the row's critical
                    # path; host flips the final signs.  The accum column
                    # collects sum_b sp, whose diag-block part cancels
                    # exactly (antisymmetric).
                    sp = sigpool.tile([P, N], f16, tag="sp")
                    nc.scalar.activation(
                        sp[:, :w], pb[:, lo:],
                        mybir.ActivationFunctionType.Sign,
                        bias=pc[:, c : c + 1], scale=-1.0,
                        accum_out=sacc[:, c : c + 1],
                    )
                    # diag block (TensorScalarPtr only encodes on DVE)
                    scrd = spool.tile([P, P], f16, tag="scrd")
                    nc.vector.scalar_tensor_tensor(
                        scrd[:],
                        tb[:, lo : lo + P],
                        tcl[:, c : c + 1],
                        sp[:, :P],
                        mybir.AluOpType.is_gt,
                        mybir.AluOpType.mult,
                        accum_out=dacc[:, c : c + 1],
                    )
                    # strictly-upper part on DVE
                    if c < NCHUNK - 1:
                        scr = spool.tile([P, N], f16, tag="scr")
                        nc.vector.scalar_tensor_tensor(
                            scr[:, : w - P],
                            tb[:, lo + P :],
                            tcl[:, c : c + 1],
                            sp[:, P:w],
                            mybir.AluOpType.is_gt,
                            mybir.AluOpType.mult,
                            accum_out=uacc[:, c : c + 1],
                        )
                # per-row free-dim reduces (overlap later rows' compute)
                nc.vector.tensor_reduce(
                    red[:, 3 * r : 3 * r + 1], dacc[:],
                    mybir.AxisListType.X, mybir.AluOpType.add,
                )
                nc.vector.tensor_reduce(
                    red[:, 3 * r + 1 : 3 * r + 2], uacc[:],
                    mybir.AxisListType.X, mybir.AluOpType.add,
                )
                nc.vector.tensor_reduce(
                    red[:, 3 * r + 2 : 3 * r + 3], sacc[:],
                    mybir.AxisListType.X, mybir.AluOpType.add,
                )
            q = apool.tile([1, 3 * ROWS_PER_CORE], f32)
            nc.gpsimd.tensor_reduce(
                q[:], red[:], mybir.AxisListType.C, mybir.AluOpType.add
            )
            nc.sync.dma_start(q_out[:], q[:])
    _split_waits(nc)
    return nc


def _get_nc():
    if "nc" not in _cache:
        _cache["nc"] = _build_nc()
    return _cache["nc"]


def _build_fast():
    """AOT-compile the 8-core sharded executable once (fast C++ dispatch)."""
    import jax
    import concourse.mybir as mybir
    from jax.sharding import Mesh, PartitionSpec

    try:
        from jax.experimental.shard_map import shard_map
    except ImportError:
        from jax import shard_map

    from concourse.bass2jax import (
        _bass_exec_p,
        install_neuronx_cc_hook,
        partition_id_tensor,
        fast_dispatch_compile,
    )

    install_neuronx_cc_hook()
    nc = _get_nc()
    partition_name = nc.partition_id_tensor.name if nc.partition_id_tensor else None

    in_names, out_names, out_avals, zero_outs = [], [], [], []
    for alloc in nc.m.functions[0].allocations:
        if not isinstance(alloc, mybir.MemoryLocationSet):
            continue
        name = alloc.memorylocations[0].name
        if alloc.kind == "ExternalInput":
            if name != partition_name:
                in_names.append(name)
        elif alloc.kind == "ExternalOutput":
            out_names.append(name)
            shape = tuple(alloc.tensor_shape)
            dtype = mybir.dt.np(alloc.dtype)
            out_avals.append(jax.core.ShapedArray(shape, dtype))
            zero_outs.append(np.zeros(shape, dtype))
    n_params = len(in_names)
    n_outs = len(out_avals)
    in_names_all = list(in_names) + list(out_names)
    if partition_name is not None:
        in_names_all.append(partition_name)
    donate = tuple(range(n_params, n_params + n_outs))

    def _body(*args):
        operands = list(args)
        if partition_name is not None:
            operands.append(partition_id_tensor())
        outs = _bass_exec_p.bind(
            *operands,
            out_avals=tuple(out_avals),
            in_names=tuple(in_names_all),
            out_names=tuple(out_names),
            lowering_input_output_aliases=(),
            sim_require_finite=True,
            sim_require_nnan=True,
            nc=nc,
        )
        return tuple(outs)

    devices = jax.devices()[:N_CORES]
    mesh = Mesh(np.asarray(devices), ("core",))
    in_specs = (PartitionSpec("core"),) * (n_params + n_outs)
    out_specs = (PartitionSpec("core"),) * len(out_names)

    global_in = [
        jax.ShapeDtypeStruct((N_CORES * 2 * ROWS_PER_CORE, N), np.float16)
        for _ in range(n_params)
    ]
    global_zero = [
        jax.ShapeDtypeStruct((N_CORES * z.shape[0], *z.shape[1:]), z.dtype)
        for z in zero_outs
    ]

    compiled = fast_dispatch_compile(
        lambda: jax.jit(
            shard_map(
                _body,
                mesh=mesh,
                in_specs=in_specs,
                out_specs=out_specs,
                check_rep=False,
            ),
            donate_argnums=donate,
            keep_unused=True,
        )
        .lower(*global_in, *global_zero)
        .compile()
    )
    zero_shapes = [(N_CORES * z.shape[0], *z.shape[1:]) for z in zero_outs]
    zero_dtypes = [z.dtype for z in zero_outs]
    zero_meta = list(zip(zero_shapes, zero_dtypes))

    # warm the executable once (first device call pays extra runtime setup)
    dummy = np.zeros((N_CORES * 2 * ROWS_PER_CORE, N), np.float16)
    out = compiled(dummy, *[np.zeros(s, d) for s, d in zero_meta])
    np.asarray(out[0])

    return compiled, in_names, zero_meta


def _get_fast():
    if "fast" not in _cache:
        _cache["fast"] = _build_fast()
    return _cache["fast"]


def _loss_from_s(s_total, n_rows):
    return np.float32(1.0 - s_total / (n_rows * PAIRS))


def _pack_inputs(pred, target):
    x = np.empty((N_CORES, 2 * ROWS_PER_CORE, N), np.float16)
    x[:, :ROWS_PER_CORE] = pred.reshape(N_CORES, ROWS_PER_CORE, N)
    x[:, ROWS_PER_CORE:] = target.reshape(N_CORES, ROWS_PER_CORE, N)
    return x.reshape(N_CORES * 2 * ROWS_PER_CORE, N)


def _combine(q):
    """q: [..., 3] per (core,row) of (diag, upper_stt, sign_sum) computed
    with sp = sign(p_a - p_b) (scale=-1 trick) -> total S (sign-flipped)."""
    q = np.asarray(q, np.float64).reshape(-1, 3)
    return float(-q[:, 0].sum() - 2.0 * q[:, 1].sum() + q[:, 2].sum())


def _kernel_fallback(pred, target):
    from concourse.bass_utils import run_bass_kernel_spmd

    nc = _get_nc()
    xg = _pack_inputs(pred, target)
    in_maps = [
        {
            "x": np.ascontiguousarray(
                xg[k * 2 * ROWS_PER_CORE : (k + 1) * 2 * ROWS_PER_CORE]
            )
        }
        for k in range(N_CORES)
    ]
    res = run_bass_kernel_spmd(nc, in_maps, core_ids=list(range(N_CORES)))
    _cache["last_perf"] = res
    s_total = _combine(np.stack([r["q"] for r in res.results]))
    return _loss_from_s(s_total, pred.shape[0])


def kernel(pred, target):
    pred = np.asarray(pred, dtype=np.float32).reshape(-1, N)
    target = np.asarray(target, dtype=np.float32).reshape(-1, N)
    n_rows = pred.shape[0]
    assert n_rows == ROWS_PER_CORE * N_CORES

    if os.environ.get("KTAU_FALLBACK", "0") == "1":
        return _kernel_fallback(pred, target)
    try:
        compiled, in_names, zero_meta = _get_fast()
    except Exception:
        return _kernel_fallback(pred, target)

    xg = _pack_inputs(pred, target)
    zeros = [np.zeros(shape, dt) for shape, dt in zero_meta]
    out = compiled(xg, *zeros)
    s_total = _combine(np.asarray(out[0]))
    return _loss_from_s(s_total, n_rows)
